# revision 50
# baseline (speedup 1.0000x reference)
"""AgentAttention TRN2 kernel: 8 cores = 4 batches x 2 head-groups.

Reference computation (B=4, T=3584, dim=1024, H=16, D=64, P=7):
  qkv = x @ W_qkv -> q,k,v [B,H,T,D]
  agent = avgpool_T(q) [B,H,P,D]
  v_agent = softmax(agent*SC @ k^T) @ v
  out_att = softmax(q*SC @ agent^T) @ v_agent
  dwc = depthwise3x3 over (H,T) of v
  out = (out_att + dwc) 'b h t d -> b t (h d)' @ W_o + b_o

Core c handles batch c//2, heads [8g, 8g+8) with g=c%2. The two partial
outputs per batch are summed on the host (+ b_o).

Structure (all engine-time figures per the TRN2 cost model):
 - q and k are never materialized. Scores contract against x directly:
     s1^T = x @ A^T,  A = (agent*SC) @ Wk^T;   s2 = x @ B,  B = Wq @ agent^T*SC
   agent = (pooled x) @ Wq, with the pooling done as free-dim reduces of
   x^T (host supplies x^T). The only large qkv-side matmul is v
   (10 head slots incl conv halo, 640 cols).
 - Stage-1 aggregation re-associated through x: va = (u1^T @ x) @ Wv,
   emitted transposed (a1^T accumulated in one PSUM bank, 8 col-groups)
   and interleaved tile-by-tile into the scores sweep.
 - The stage-2 attention output matmul accumulates into the depthwise
   conv PSUM tile as a 10th accumulation step.
 - DMA transfers and HWDGE generation are each globally serialized in
   the cost model, so: weights are packed host-side into one blob DMA,
   x^T arrives as 32 column-slab DMAs ordered so PE can start after the
   first slab, x-natural streams through a rolling pool during the
   scores sweep, and the output is written bf16, one DMA per token tile
   on the software-DGE (Pool) path to keep HWDGE free.
"""

import numpy as np
import ml_dtypes

import concourse.bass as bass
import concourse.bacc as bacc
import concourse.mybir as mybir
import concourse.tile as tile
from concourse.bass import ts, ds
from concourse import bass_utils

F32 = mybir.dt.float32
BF16 = mybir.dt.bfloat16
AX = mybir.AxisListType
AF = mybir.ActivationFunctionType
AL = mybir.AluOpType

T, DIM, D, P = 3584, 1024, 64, 7
HL = 8                      # local heads per core
SC = D ** -0.5
NT = T // 128               # 28 token tiles of 128
NCH = T // 512              # 7 chunks of 512
TPAD = T + 2                # vT padded with one zero col each side

# blob layout (bf16, per-partition column offsets)
OWQN, OWKT, OWQT, OWOP, OTAPS = 0, 4096, 8192, 12288, 16384
OCBT = OTAPS + 9 * 128      # conv_b tiled row, replicated on all partitions
OIDB = OCBT + 512           # bf16 identity (for p2 transposes)
BLOBW = OIDB + 128          # 18176


def build_nc(skip=()):
    # Bacc (not plain Bass): its compile() runs generate_event_semaphores,
    # which splits multi-wait sync_info into InstEventSemaphore -- TRN2
    # instructions can carry at most one embedded wait.
    nc = bacc.Bacc("TRN2", target_bir_lowering=False)

    xbT = nc.dram_tensor("xbT", [DIM, T], BF16, kind="ExternalInput")
    xb = nc.dram_tensor("xb", [T, DIM], BF16, kind="ExternalInput")
    wvpa = nc.dram_tensor("wvpa", [128, 1024], BF16, kind="ExternalInput")
    wvpc = [nc.dram_tensor(f"wvpc{c}", [128, 1024], BF16,
                           kind="ExternalInput") for c in range(4)]
    blob = nc.dram_tensor("blob", [128, BLOBW], BF16, kind="ExternalInput")
    fblob = nc.dram_tensor("fblob", [128, 138], F32, kind="ExternalInput")
    outp = nc.dram_tensor("outp", [T, DIM], BF16, kind="ExternalOutput")

    with tile.TileContext(nc) as tc:
        _emit(nc, tc, xbT, xb, wvpa, wvpc, blob, fblob, outp)
    nc.compile()
    return nc


def _copy(eng, out, in_):
    if hasattr(eng, "activation"):
        eng.copy(out, in_)
    else:
        eng.tensor_copy(out, in_)


def _emit(nc, tc, xbT, xb, wvpa, wvpc, blob, fblob, outp):
    import contextlib
    ctx = contextlib.ExitStack()
    with ctx:
        # ---- constants + weight blob ----------------------------------
        pconst = ctx.enter_context(tc.tile_pool(name="const", bufs=1))
        fb_sb = pconst.tile([128, 138], F32, name="fb", tag="fb")
        nc.sync.dma_start(fb_sb[:], fblob[:])
        idf_sb = fb_sb[:, 0:128]
        wcol = fb_sb[:, 129:138]    # per-partition dwc tap weights
        ones_sb = pconst.tile([128, 1], BF16, name="ones", tag="ones")
        nc.vector.memset(ones_sb[:], 1.0)

        pwvp = ctx.enter_context(tc.tile_pool(name="wvp", bufs=1))
        wvpa_sb = pwvp.tile([128, 1024], BF16, name="wvpa", tag="wvpa")
        nc.sync.dma_start(wvpa_sb[:], wvpa[:])
        wvpa_v = wvpa_sb.rearrange("p (k c) -> p k c", c=128)
        wvpc_sb = [pwvp.tile([128, 1024], BF16, name=f"wvpc{c}",
                             tag=f"wvpc{c}") for c in range(4)]
        wvpc_v = [t.rearrange("p (k c) -> p k c", c=128) for t in wvpc_sb]

        def wv_lhs(kk, cg):
            if cg == 0:
                return wvpa_v[:, kk, :]
            return wvpc_v[cg - 1][:, kk, :]

        psmall = ctx.enter_context(tc.tile_pool(name="small", bufs=1))
        xsumT = [psmall.tile([128, P], F32, name=f"xsT{k}", tag=f"xsT{k}")
                 for k in range(8)]
        xsumB = [psmall.tile([128, P], BF16, name=f"xsB{k}", tag=f"xsB{k}")
                 for k in range(8)]
        R = [psmall.tile([128, 2 * P], BF16, name=f"R{j}", tag=f"R{j}")
             for j in range(4)]
        AB = [psmall.tile([128, 112], BF16, name=f"AB{k}", tag=f"AB{k}")
              for k in range(8)]
        u1T = psmall.tile([128, NT * 56], BF16, name="u1T", tag="u1T")
        p2T = psmall.tile([56, T], BF16, name="p2T", tag="p2T")
        a1sb = psmall.tile([128, 8 * 56], BF16, name="a1sb", tag="a1sb")
        vabd = psmall.tile([56, 512], BF16, name="vabd", tag="vabd")
        rec1 = psmall.tile([56, 1], F32, name="rec1", tag="rec1")

        # ---- vT: v (10 head slots incl halo) transposed, bf16, t-padded
        pvT = ctx.enter_context(tc.tile_pool(name="vT", bufs=1))
        vT = [pvT.tile([128, TPAD], BF16, name=f"vT{j}", tag=f"vT{j}")
              for j in range(5)]
        for j in range(5):
            nc.vector.memset(vT[j][:, 0:1], 0.0)
            nc.vector.memset(vT[j][:, TPAD - 1:TPAD], 0.0)

        # pools that outlive xT must be created before it (LIFO release)
        pblob = ctx.enter_context(tc.tile_pool(name="blob", bufs=1))
        pvO = ctx.enter_context(tc.tile_pool(name="vO", bufs=1))
        pxn = ctx.enter_context(tc.tile_pool(name="xnat", bufs=12))

        # ---- phase 1: xT slab loads + v matmul ------------------------
        # xT arrives in 4 column slabs x 8 kk tiles; v-matmul chains are
        # ordered by slab so PE starts once slab 0 lands.
        import contextlib as _cl
        xT_stack = _cl.ExitStack()
        pxT = xT_stack.enter_context(tc.tile_pool(name="xT", bufs=1))
        xTb = pxT.tile([128, 8 * T], BF16, name="xTb", tag="xTb")

        def xT(kk):
            return xTb[:, ds(kk * T, T)]

        # DMA order: each xT slab followed by one wv column-group tensor;
        # the chain order below matches these arrival times.
        SLABS = [(0, 512), (512, 512), (1024, 1024), (2048, 1024),
                 (3072, 512)]
        for si, (off, w) in enumerate(SLABS):
            for kk in range(8):
                nc.sync.dma_start(
                    xTb[:, ds(kk * T + off, w)],
                    xbT[ts(kk, 128), ds(off, w)])
            if si < 4:
                nc.sync.dma_start(wvpc_sb[si][:], wvpc[si][:])

        CHAIN_ORDER = (
            [(0, 0), (1, 0)] +
            [(2, 0), (0, 1), (1, 1), (2, 1)] +
            [(3, 0), (3, 1), (0, 2), (1, 2), (2, 2), (3, 2),
             (0, 3), (1, 3), (2, 3), (3, 3)] +
            [(4, 0), (4, 1), (4, 2), (4, 3)] +
            [(cg, ch) for ch in (4, 5) for cg in range(5)] +
            [(cg, 6) for cg in range(5)])

        with tc.tile_pool(name="mmps", bufs=4, space="PSUM") as pmm:
            done_ch = set()
            for nmm, (cg, ch) in enumerate(CHAIN_ORDER):
                pm = pmm.tile([128, 512], F32, name="mm", tag="mm")
                for kk in range(8):
                    nc.tensor.matmul(
                        pm[:], wv_lhs(kk, cg),
                        xT(kk)[:, ds(ch * 512, 512)],
                        start=(kk == 0), stop=(kk == 7),
                    )
                eng = nc.scalar if nmm % 2 else nc.vector
                _copy(eng, vT[cg][:, ds(1 + ch * 512, 512)], pm[:])
                # xsum reduces once per chunk (keeps DVE current so the
                # agent/AB chain isn't stuck behind late vT copies)
                if ch not in done_ch:
                    done_ch.add(ch)
                    for kk in range(8):
                        nc.vector.reduce_sum(
                            xsumT[kk][:, ch:ch + 1],
                            xT(kk)[:, ds(ch * 512, 512)], axis=AX.X)

        # weight blob (queued on sync after the xT slabs)
        blob_sb = pblob.tile([128, BLOBW], BF16, name="blob", tag="blob")
        nc.sync.dma_start(blob_sb[:], blob[:])
        wqn_v = blob_sb[:, OWQN:OWQN + 4096].rearrange(
            "p (k c) -> p k c", c=512)
        wkt_v = blob_sb[:, OWKT:OWKT + 4096].rearrange(
            "p (k c) -> p k c", c=1024)
        wqt_v = blob_sb[:, OWQT:OWQT + 4096].rearrange(
            "p (k c) -> p k c", c=1024)
        wop_v = blob_sb[:, OWOP:OWOP + 4096].rearrange(
            "p (k c) -> p k c", c=1024)
        taps_v = blob_sb[:, OTAPS:OTAPS + 1152].rearrange(
            "p (k c) -> p k c", c=128)
        idb_v = blob_sb[:, OIDB:OIDB + 128]

        # xsum -> bf16 for the agent matmuls
        for kk in range(8):
            nc.scalar.copy(xsumB[kk][:], xsumT[kk][:])

        # ---- agent + A/B formation ------------------------------------
        with tc.tile_pool(name="agps", bufs=1, space="PSUM") as pagp, \
             tc.tile_pool(name="abps", bufs=4, space="PSUM") as pabp:
            agJ = [pagp.tile([128, P], F32, name=f"agJ{j}", tag=f"agJ{j}")
                   for j in range(4)]
            for j in range(4):
                for kk in range(8):
                    nc.tensor.matmul(
                        agJ[j][:], wqn_v[:, kk, ds(j * 128, 128)],
                        xsumB[kk][:], start=(kk == 0), stop=(kk == 7))
            # R[j]: block-diagonal agent^T * SC/512, bf16 [128, 14]
            for j in range(4):
                nc.vector.memset(R[j][:], 0.0)
                nc.scalar.activation(
                    R[j][0:64, 0:P], agJ[j][0:64, :], AF.Copy,
                    scale=SC / 512.0)
                nc.scalar.activation(
                    R[j][64:128, P:2 * P], agJ[j][64:128, :], AF.Copy,
                    scale=SC / 512.0)
            # AB[kk] = [A^T | B] slice [128 m, 112]
            for kk in range(8):
                pab = pabp.tile([128, 112], F32, name="ab", tag="ab")
                for j in range(4):
                    nc.tensor.matmul(
                        pab[:, ts(j, 14)], wkt_v[:, j, ts(kk, 128)],
                        R[j][:], start=True, stop=True,
                        skip_group_check=True)
                    nc.tensor.matmul(
                        pab[:, ds(56 + j * 14, 14)],
                        wqt_v[:, j, ts(kk, 128)],
                        R[j][:], start=True, stop=True,
                        skip_group_check=True)
                eng = nc.scalar if kk % 2 else nc.vector
                _copy(eng, AB[kk][:], pab[:])

        # vO (odd-aligned slot pairs): SBUF->SBUF partition-shift DMAs.
        # Emitted mid-scores so they don't starve the xn stream.
        vO = [pvO.tile([128, TPAD], BF16, name=f"vO{i}", tag=f"vO{i}")
              for i in range(4)]

        def emit_vO():
            for i in range(4):
                nc.scalar.dma_start(vO[i][0:64, :], vT[i][64:128, :])
                nc.scalar.dma_start(vO[i][64:128, :], vT[i + 1][0:64, :])

        # ---- scores sweep + interleaved stage-1 aggregation -----------
        # Per tile tt: scores matmuls + exp/softmax; the agg matmuls and
        # p2 transpose for tile tt-1 are emitted one iteration later so
        # PE never waits on the exp/softmax chain.
        xn_tiles = {}

        def load_xn(tt):
            xn = pxn.tile([128, DIM], BF16, name="xn", tag="xn")
            nc.sync.dma_start(xn[:], xb[ts(tt, 128), :])
            xn_tiles[tt] = xn

        for tt in range(12):
            load_xn(tt)

        with tc.tile_pool(name="sps", bufs=3, space="PSUM") as pps, \
             tc.tile_pool(name="trps", bufs=2, space="PSUM") as ptr, \
             tc.tile_pool(name="a1ps", bufs=1, space="PSUM") as pa1p, \
             tc.tile_pool(name="csps", bufs=1, space="PSUM") as pcsp, \
             tc.tile_pool(name="stmp", bufs=4) as pst:
            a1T = pa1p.tile([128, 8 * 56], F32, name="a1T", tag="a1T")
            pcs = pcsp.tile([56, 1], F32, name="cs", tag="cs")
            p2fs = {}

            def emit_agg(tt):
                for kk in range(8):
                    nc.tensor.matmul(
                        a1T[:, ts(kk, 56)],
                        xn_tiles[tt][:, ts(kk, 128)], u1T[:, ts(tt, 56)],
                        start=(tt == 0), stop=(tt == NT - 1),
                        skip_group_check=True)
                del xn_tiles[tt]
                nc.tensor.matmul(pcs[:], u1T[:, ts(tt, 56)], ones_sb[:],
                                 start=(tt == 0), stop=(tt == NT - 1))

            def emit_p2t(tt):
                ptt = ptr.tile([56, 128], BF16, name="p2t", tag="p2t")
                nc.tensor.transpose(ptt[:], p2fs.pop(tt)[:], idb_v)
                nc.any.tensor_copy(p2T[:, ts(tt, 128)], ptt[:])

            for tt in range(NT + 2):
                if tt == 14:
                    emit_vO()
                if tt < NT:
                    if tt + 12 < NT:
                        load_xn(tt + 12)
                    ps = pps.tile([128, 112], F32, name="s", tag="s")
                    for kk in range(8):
                        nc.tensor.matmul(
                            ps[:], xT(kk)[:, ts(tt, 128)], AB[kk][:],
                            start=(kk == 0), stop=(kk == 7))
                if tt > 1:
                    emit_agg(tt - 2)
                    emit_p2t(tt - 2)
                if tt < NT:
                    nc.scalar.activation(
                        u1T[:, ts(tt, 56)], ps[:, 0:56], AF.Exp)
                    u2 = pst.tile([128, 56], F32, name="u2", tag="u2")
                    nc.scalar.activation(u2[:], ps[:, 56:112], AF.Exp)
                    rs = pst.tile([128, 8], F32, name="rs", tag="rs")
                    nc.vector.reduce_sum(
                        rs[:], u2.rearrange("p (h q) -> p h q", q=P),
                        axis=AX.X)
                    nc.vector.reciprocal(rs[:], rs[:])
                    p2f = pst.tile([128, 56], BF16, name="p2f", tag="p2f")
                    nc.vector.tensor_tensor(
                        out=p2f.rearrange("p (h q) -> p h q", q=P),
                        in0=u2.rearrange("p (h q) -> p h q", q=P),
                        in1=rs[:, :, None].broadcast_to([128, 8, P]),
                        op=AL.mult,
                    )
                    p2fs[tt] = p2f

            nc.vector.reciprocal(rec1[:], pcs[:])
            nc.scalar.copy(a1sb[:], a1T[:])

        xT_stack.close()  # free xTb SBUF

        # ---- dwc + v_agent + attention output -------------------------
        # conv_b is folded into vabd: p2 rows sum to 1 per (token, head),
        # so adding conv_b to the vabd diag blocks makes the fused
        # attention matmul contribute exactly out_att + conv_b.
        # 7 of 9 taps + the stage-2 attention matmul accumulate in PSUM;
        # the two remaining vO taps are fused DVE ops on the extraction
        # path. The first two tap chains are emitted before the va chain
        # so PE stays busy while Act/DVE produce vabd.
        pY = ctx.enter_context(tc.tile_pool(name="Ypool", bufs=1))
        Y = [pY.tile([128, T], BF16, name=f"Y{i}", tag=f"Y{i}")
             for i in range(4)]

        with tc.tile_pool(name="dwcps", bufs=4, space="PSUM") as pdw:
            def emit_taps(i, tc7):
                src_by_kh = (vT[i], vO[i], vT[i + 1])
                off = 1 + tc7 * 512
                pd = pdw.tile([128, 512], F32, name="dwc", tag="dwc")
                first = True
                for kh, kt in ((0, 0), (0, 1), (0, 2), (1, 2),
                               (2, 0), (2, 1), (2, 2)):
                    nc.tensor.matmul(
                        pd[:], taps_v[:, kh * 3 + kt, :],
                        src_by_kh[kh][:, ds(off + kt - 1, 512)],
                        start=first, stop=False)
                    first = False
                return pd

            def emit_finish(i, tc7, pd):
                off = 1 + tc7 * 512
                # stage-2 attention output (+conv_b): last accumulation
                nc.tensor.matmul(
                    pd[:], vabd[:, ts(i, 128)], p2T[:, ts(tc7, 512)],
                    start=False, stop=True)
                yslc = Y[i][:, ts(tc7, 512)]
                nc.vector.scalar_tensor_tensor(
                    out=yslc, in0=vO[i][:, ds(off, 512)],
                    scalar=wcol[:, 4:5], in1=pd[:],
                    op0=AL.mult, op1=AL.add)
                nc.vector.scalar_tensor_tensor(
                    out=yslc, in0=vO[i][:, ds(off - 1, 512)],
                    scalar=wcol[:, 3:4], in1=yslc,
                    op0=AL.mult, op1=AL.add)

            pd00 = emit_taps(0, 0)
            pd01 = emit_taps(0, 1)

            with tc.tile_pool(name="vaps", bufs=1, space="PSUM") as pvap, \
                 tc.tile_pool(name="vtmp", bufs=1) as pvt:
                pva = pvap.tile([56, 512], F32, name="va", tag="va")
                va_rhs = [(0, 64, wvpa_v, 64, 64)] + \
                    [(64 + 128 * c, 128, wvpc_v[c], 0, 128)
                     for c in range(3)] + [(448, 64, wvpc_v[3], 0, 64)]
                for kk in range(8):
                    for (o, w, src, so, sw) in va_rhs:
                        nc.tensor.matmul(
                            pva[:, ds(o, w)], a1sb[:, ts(kk, 56)],
                            src[:, kk, ds(so, sw)],
                            start=(kk == 0), stop=(kk == 7),
                            skip_group_check=True)
                van = pvt.tile([56, 512], BF16, name="van", tag="van")
                nc.vector.scalar_tensor_tensor(
                    out=van[:], in0=pva[:], scalar=rec1[:],
                    in1=blob_sb[0:56, OCBT:OCBT + 512],
                    op0=AL.mult, op1=AL.add,
                )
                nc.vector.memset(vabd[:], 0.0)
                for h in range(HL):
                    nc.sync.dma_start(
                        vabd[ds(P * h, P), ds(64 * h, 64)],
                        van[ds(P * h, P), ds(64 * h, 64)])

            emit_finish(0, 0, pd00)
            emit_finish(0, 1, pd01)
            for i in range(4):
                for tc7 in range(2 if i == 0 else 0, NCH):
                    pd = emit_taps(i, tc7)
                    emit_finish(i, tc7, pd)

        # ---- output projection: out = Y^T @ Wo ------------------------
        with tc.tile_pool(name="ostage", bufs=3) as pos, \
             tc.tile_pool(name="ops", bufs=4, space="PSUM") as pop:
            for tt in range(NT):
                po = pop.tile([128, DIM], F32, name="o", tag="o")
                for half in range(2):
                    for k in range(4):
                        nc.tensor.matmul(
                            po[:, ts(half, 512)],
                            Y[k][:, ts(tt, 128)],
                            wop_v[:, k, ts(half, 512)],
                            start=(k == 0), stop=(k == 3))
                ost = pos.tile([128, DIM], BF16, name="ost", tag="ost")
                if tt < NT - 2:
                    _copy(nc.scalar if tt % 2 else nc.vector, ost[:], po[:])
                    nc.gpsimd.dma_start(outp[ts(tt, 128), :], ost[:])
                elif tt == NT - 2:
                    nc.scalar.copy(ost[:], po[:])
                    nc.scalar.dma_start(outp[ts(tt, 128), :], ost[:])
                else:   # last tile: split halves, low-latency HWDGE path
                    nc.vector.tensor_copy(ost[:, 0:512], po[:, 0:512])
                    nc.scalar.copy(ost[:, 512:DIM], po[:, 512:DIM])
                    nc.sync.dma_start(outp[ts(tt, 128), 0:512],
                                      ost[:, 0:512])
                    nc.scalar.dma_start(outp[ts(tt, 128), 512:DIM],
                                        ost[:, 512:DIM])


_NC_CACHE = None


def _get_nc():
    global _NC_CACHE
    if _NC_CACHE is None:
        _NC_CACHE = build_nc()
    return _NC_CACHE


def _prep_core_inputs(x, W_qkv, W_o, conv_w, conv_b):
    bf = ml_dtypes.bfloat16
    ins = []
    # taps[kh*3+kt] = kron(I2, diag(conv_w[:, 0, kh, kt]))
    taps_np = np.zeros((9, 128, 128), dtype=np.float32)
    cw = np.asarray(conv_w, np.float32)
    for kh in range(3):
        for kt in range(3):
            dg = np.diag(cw[:, 0, kh, kt])
            taps_np[kh * 3 + kt, 0:64, 0:64] = dg
            taps_np[kh * 3 + kt, 64:128, 64:128] = dg
    fblob = np.zeros((128, 138), np.float32)
    fblob[:, 0:128] = np.eye(128, dtype=np.float32)
    fblob[:, 128] = np.tile(conv_b, 2)
    for kh in range(3):
        for kt in range(3):
            fblob[:, 129 + kh * 3 + kt] = np.tile(cw[:, 0, kh, kt], 2)

    def pack(w, k):  # [k*128, c] -> [128, k, c]
        c = w.shape[1]
        return np.ascontiguousarray(
            w.reshape(k, 128, c).transpose(1, 0, 2))

    for c in range(8):
        b, g = c // 2, c % 2
        wq = W_qkv[:, 512 * g:512 * g + 512]
        wk = W_qkv[:, 1024 + 512 * g:1024 + 512 * g + 512]
        wv10 = np.zeros((DIM, 640), np.float32)
        for s in range(10):
            h = 8 * g - 1 + s
            if 0 <= h < 16:
                wv10[:, 64 * s:64 * s + 64] = \
                    W_qkv[:, 2048 + 64 * h:2048 + 64 * h + 64]
        blob = np.empty((128, BLOBW), np.float32)
        blob[:, OWQN:OWQN + 4096] = pack(wq, 8).reshape(128, 4096)
        blob[:, OWKT:OWKT + 4096] = pack(
            np.ascontiguousarray(wk.T), 4).reshape(128, 4096)
        blob[:, OWQT:OWQT + 4096] = pack(
            np.ascontiguousarray(wq.T), 4).reshape(128, 4096)
        blob[:, OWOP:OWOP + 4096] = pack(
            np.ascontiguousarray(W_o[512 * g:512 * g + 512, :]),
            4).reshape(128, 4096)
        blob[:, OTAPS:OTAPS + 1152] = taps_np.transpose(1, 0, 2).reshape(
            128, 1152)
        blob[:, OCBT:OCBT + 512] = np.tile(conv_b, 8)[None, :]
        blob[:, OIDB:OIDB + 128] = np.eye(128, dtype=np.float32)
        ins.append({
            "xbT": np.ascontiguousarray(x[b].T).astype(bf),
            "xb": np.ascontiguousarray(x[b]).astype(bf),
            "wvpa": pack(wv10[:, 0:128], 8).reshape(128, 1024).astype(bf),
            **{f"wvpc{c}": pack(wv10[:, 128 * (c + 1):128 * (c + 2)],
                                8).reshape(128, 1024).astype(bf)
               for c in range(4)},
            "blob": blob.astype(bf),
            "fblob": fblob,
        })
    return ins


def kernel(x, W_qkv, W_o, b_o, conv_w, conv_b, _run_kwargs=None):
    x = np.asarray(x, np.float32)
    W_qkv = np.asarray(W_qkv, np.float32)
    W_o = np.asarray(W_o, np.float32)
    b_o = np.asarray(b_o, np.float32)
    conv_w = np.asarray(conv_w, np.float32)
    conv_b = np.asarray(conv_b, np.float32)

    ins = _prep_core_inputs(x, W_qkv, W_o, conv_w, conv_b)

    nc = _get_nc()
    res = bass_utils.run_bass_kernel_spmd(
        nc, ins, core_ids=list(range(8)), **(_run_kwargs or {}))
    outs = [r["outp"] for r in res.results]
    B = x.shape[0]
    full = np.empty((B, T, DIM), np.float32)
    for b in range(B):
        full[b] = (outs[2 * b].astype(np.float32)
                   + outs[2 * b + 1].astype(np.float32) + b_o[None, :])
    if _run_kwargs:
        kernel.last_results = res
    return full
